# revision 1
# baseline (speedup 1.0000x reference)
"""MoE layer (hash-routed, top-k=2, E=8 experts) on 8 Trainium2 NeuronCores.

Strategy: expert-parallel. Core e holds expert e's weights (W1[e], W2[e]).
The host routes: for each expert, gather the distinct tokens assigned to it
(a token contributes once per distinct expert), transpose the gathered
activations to [D, C] so the device never has to transpose, run a dense
2-layer MLP per core, then scatter-add the per-expert outputs back and
divide by k.

All matmul operands are bf16 (halves DMA traffic + SBUF footprint; rel err
~3e-3 vs the 2e-2 gate); PSUM/y accumulate in f32. Device kernel (per core),
C = token capacity (padded, multiple of 512):

  for each quarter q of H (HQ=1024 columns):
    layer1: H1T[h, tok] = relu(W1q^T @ XT + b1q)   (PSUM-accum over d-tiles,
            two token-chunk chains interleaved so consecutive matmuls reuse
            the stationary weights and alternate PSUM banks)
    layer2: Y[tok, d]  += H1T^T @ W2q              (PSUM-accum over h-tiles,
            dc=0/1 chains share the stationary h-tile; SBUF f32 accum over q)

Quarter 0's layer 1 runs chunk-outer so the PE starts once the first x
chunk + first W1 h-tile land. DMA discipline (measured on hw): each
dma_start costs ~640ns on a shared serialized push path, while a single
large instruction fans its lines out across all 16 DMA queues at full HBM
bandwidth, and completion is signaled per instruction. So transfers are
batched into as few instructions as possible, split only where the
completion order must match the PE's consumption order (W1(q0) in 1/3/4
h-tile batches), and bulk prefetches are pushed from the scalar queue only
once it reaches that point of the activation stream (so they cannot steal
bandwidth from the critical prologue).
"""

import math
import numpy as np
import ml_dtypes

import concourse.bass as bass
import concourse.mybir as mybir
import concourse.tile as tile
from concourse import bacc
from concourse.bass_utils import run_bass_kernel_spmd

dt = mybir.dt
BF16 = np.dtype(ml_dtypes.bfloat16)

B, S, D, H, E, NCORES = 4, 1024, 1024, 4096, 8, 8
HQ = 1024                      # h-quarter width
KT = D // 128                  # 8 contraction tiles (d)
HTQ = HQ // 128                # 8 h-tiles per quarter
NQ = H // HQ                   # 4 quarters
NWARM = 40                     # PE clock-ramp warmup matmuls (cover the
                               # DMA prologue so the first real matmuls
                               # run at full clock)

_BUILD_CACHE: dict = {}


def _chunks512(C):
    return [(c0, min(512, C - c0)) for c0 in range(0, C, 512)]


def build_nc(C: int):
    """Build + compile the per-core Bass program for token capacity C."""
    assert C % 512 == 0
    TT = C // 128
    n_chunks = _chunks512(C)
    nch = len(n_chunks)

    nc = bacc.Bacc(
        "TRN2",
        target_bir_lowering=False,
        debug=False,
        num_devices=NCORES,
    )

    xt_d = nc.dram_tensor("xt", [128, KT * C], dt.bfloat16, kind="ExternalInput")
    w1_d = nc.dram_tensor(
        "w1", [128, NQ * HTQ * KT * 128], dt.bfloat16, kind="ExternalInput"
    )
    b1_d = nc.dram_tensor("b1", [H], dt.float32, kind="ExternalInput")
    w2_d = nc.dram_tensor("w2", [128, NQ * HTQ * D], dt.bfloat16, kind="ExternalInput")
    y_d = nc.dram_tensor("y", [C, D], dt.float32, kind="ExternalOutput")

    xt_v = xt_d.ap().rearrange("p (kt c) -> p kt c", kt=KT)
    w1_v = w1_d.ap().rearrange(
        "p (q ht kt j) -> p q ht kt j", q=NQ, ht=HTQ, kt=KT
    )
    w2_v = w2_d.ap().rearrange("p (q ht d) -> p q ht d", q=NQ, ht=HTQ)
    b1_v = b1_d.ap().rearrange("(ht p) -> p ht", p=128)
    y_v = y_d.ap().rearrange("(tt p) d -> p tt d", p=128)

    with tile.TileContext(nc) as tc:
        with (
            tc.tile_pool(name="xt", bufs=1) as xt_pool,
            tc.tile_pool(name="b1", bufs=1) as b1_pool,
            tc.tile_pool(name="y", bufs=1) as y_pool,
            tc.tile_pool(name="w1q", bufs=2) as w1_pool,
            tc.tile_pool(name="w2q", bufs=2) as w2_pool,
            tc.tile_pool(name="h1q", bufs=2) as h1_pool,
            tc.tile_pool(name="warm", bufs=1) as warm_pool,
            tc.tile_pool(name="gd", bufs=1) as gd_pool,
            tc.tile_pool(name="ps1", bufs=nch + 1, space="PSUM") as ps1_pool,
            tc.tile_pool(name="ps2", bufs=4, space="PSUM") as ps2_pool,
        ):
            # PE warm-up: dependency-free bf16 matmuls issued during the DMA
            # prologue so the clock ramps toward 2.4 GHz before real work.
            wt = warm_pool.tile([128, 256], dt.bfloat16)
            nc.vector.memset(wt[:], 0.0)
            wps = ps2_pool.tile([128, 256], dt.float32, tag="ps2")
            for _ in range(NWARM):
                nc.tensor.matmul(wps[:], wt[:, :128], wt[:], start=True, stop=True)

            b1t = b1_pool.tile([128, H // 128], dt.float32)
            nc.gpsimd.dma_start(b1t[:], b1_v)

            xt = xt_pool.tile([128, KT, C], dt.bfloat16)
            y = y_pool.tile([128, TT, 1024], dt.float32)
            gdum = gd_pool.tile([128, 8], dt.float32, name="gdum")

            # critical prologue on the two fast hwdge queues only (gpsimd
            # pushes are slow and its ring backs up): sync carries w1(q0)
            # ht0-1 + x chunk-0 kt0-3, scalar carries x chunk-0 kt4-7
            # wave 0, interleaved in need-order on sync; chunk-0 kt pieces
            # split across all three queues. w1 ht4-7 are NOT needed until
            # the second half-sweep (~14us later), so they stream lazily.
            w1c = w1_pool.tile([128, HTQ, KT, 128], dt.bfloat16, tag="w1q")
            nc.sync.dma_start(w1c[:, 0], w1_v[:, 0, 0])
            nc.sync.dma_start(xt[:, 0, 0:512], xt_v[:, 0, 0:512])
            nc.sync.dma_start(w1c[:, 1], w1_v[:, 0, 1])
            nc.sync.dma_start(xt[:, 1, 0:512], xt_v[:, 1, 0:512])
            nc.sync.dma_start(w1c[:, 2], w1_v[:, 0, 2])
            nc.sync.dma_start(w1c[:, 3], w1_v[:, 0, 3])
            for kt in (2, 3, 4, 5):
                nc.scalar.dma_start(xt[:, kt, 0:512], xt_v[:, kt, 0:512])
            for kt in (6, 7):
                nc.gpsimd.dma_start(xt[:, kt, 0:512], xt_v[:, kt, 0:512])

            def load_w1q(q, eng=None, hts=None):
                t = w1_pool.tile([128, HTQ, KT, 128], dt.bfloat16, tag="w1q")
                for ht in hts if hts is not None else range(HTQ):
                    (eng or nc.sync).dma_start(t[:, ht], w1_v[:, q, ht])
                return t

            def load_w2q(q, eng=None):
                t = w2_pool.tile([128, HTQ, 1024], dt.bfloat16, tag="w2q")
                for ht in range(HTQ):
                    (eng or nc.sync).dma_start(t[:, ht], w2_v[:, q, ht])
                return t

            w2c = None  # loaded from the scalar queue during L1(q0)
            w1n = None
            w2n = None

            for q in range(NQ):
                h1 = h1_pool.tile([128, HTQ, C], dt.bfloat16, tag="h1")

                # ---- layer 1: H1T[h, tok] = relu(W1q^T @ XT + b1) ----
                if q == 0:
                    # two half-sweeps over ht (0-3 then 4-7), chunk-outer
                    # inside each: the first sweep only needs w1 ht0-3 + x,
                    # so the PE starts early while w1 ht4-7 / x chunk-1 /
                    # W2(q0) stream lazily behind activations
                    q0_phases = [
                        (hb, c0, n)
                        for hb in range(2)
                        for (c0, n) in n_chunks
                    ]
                    for pi, (hb, c0, n) in enumerate(q0_phases):
                        for ht in range(hb * 4, hb * 4 + 4):
                            ps = ps1_pool.tile([128, 512], dt.float32, tag="ps1")
                            for kt in range(KT):
                                nc.tensor.matmul(
                                    ps[:, :n],
                                    w1c[:, ht, kt],
                                    xt[:, kt, c0 : c0 + n],
                                    start=(kt == 0),
                                    stop=(kt == KT - 1),
                                )
                            nc.scalar.activation(
                                h1[:, ht, c0 : c0 + n],
                                ps[:, :n],
                                mybir.ActivationFunctionType.Relu,
                                bias=b1t[:, q * HTQ + ht : q * HTQ + ht + 1],
                            )
                            if pi == 0 and ht == 0:
                                # lazy streams, gated on the first act:
                                # x chunk-1 first half + w1 ht6-7 (gpsimd,
                                # dummy-read gate), w1 ht4-5 (scalar)
                                nc.gpsimd.tensor_copy(
                                    gdum[:, 0:1], h1[:, 0, 0:1]
                                )
                                for ci, (c1, n1) in enumerate(n_chunks[1:]):
                                    for kt in range(KT if ci else KT // 2):
                                        nc.gpsimd.dma_start(
                                            xt[:, kt, c1 : c1 + n1],
                                            xt_v[:, kt, c1 : c1 + n1],
                                        )
                                for w_ht in (6, 7):
                                    nc.gpsimd.dma_start(
                                        w1c[:, w_ht], w1_v[:, 0, w_ht]
                                    )
                                for w_ht in (4, 5):
                                    nc.scalar.dma_start(
                                        w1c[:, w_ht], w1_v[:, 0, w_ht]
                                    )
                            if pi == 0 and ht == 1 and len(n_chunks) > 1:
                                # second half of x chunk-1 rides the scalar
                                # queue behind this activation
                                c1, n1 = n_chunks[1]
                                for kt in range(KT // 2, KT):
                                    nc.scalar.dma_start(
                                        xt[:, kt, c1 : c1 + n1],
                                        xt_v[:, kt, c1 : c1 + n1],
                                    )
                        # bulk prefetches behind the activation stream
                        if pi == len(n_chunks) - 1:
                            w2c = load_w2q(0, eng=nc.scalar)
                        elif pi == len(n_chunks):
                            w1n = load_w1q(1, eng=nc.scalar)
                            w2n = load_w2q(1, eng=nc.scalar)
                    if w1n is None:
                        w1n = load_w1q(1, eng=nc.scalar)
                        w2n = load_w2q(1, eng=nc.scalar)
                else:
                    # chunk chains interleaved: stationary w1 tile feeds nch
                    # consecutive matmuls, PSUM banks alternate
                    for ht in range(HTQ):
                        pss = [
                            ps1_pool.tile(
                                [128, 512], dt.float32, tag="ps1", name=f"psl1_{ci}"
                            )
                            for ci in range(nch)
                        ]
                        for kt in range(KT):
                            for ci, (c0, n) in enumerate(n_chunks):
                                nc.tensor.matmul(
                                    pss[ci][:, :n],
                                    w1c[:, ht, kt],
                                    xt[:, kt, c0 : c0 + n],
                                    start=(kt == 0),
                                    stop=(kt == KT - 1),
                                )
                        for ci, (c0, n) in enumerate(n_chunks):
                            nc.scalar.activation(
                                h1[:, ht, c0 : c0 + n],
                                pss[ci][:, :n],
                                mybir.ActivationFunctionType.Relu,
                                bias=b1t[:, q * HTQ + ht : q * HTQ + ht + 1],
                            )

                # w1 for q+2 now that w1(q) is done; sync-queue order keeps
                # this behind the already-queued transfers
                if q + 2 < NQ:
                    w1c = w1n
                    w1n = load_w1q(q + 2)
                elif q + 1 < NQ:
                    w1c = w1n

                # ---- layer 2: Y[tok, d] += H1T^T @ W2q ----
                for tq in range(TT):
                    psA = ps2_pool.tile([128, 512], dt.float32, tag="ps2")
                    psB = ps2_pool.tile([128, 512], dt.float32, tag="ps2")
                    for ht in range(HTQ):
                        st, sp = ht == 0, ht == HTQ - 1
                        lhs = h1[:, ht, tq * 128 : (tq + 1) * 128]
                        nc.tensor.matmul(
                            psA[:], lhs, w2c[:, ht, 0:512], start=st, stop=sp
                        )
                        nc.tensor.matmul(
                            psB[:], lhs, w2c[:, ht, 512:1024], start=st, stop=sp
                        )
                    ys0 = y[:, tq, 0:512]
                    ys1 = y[:, tq, 512:1024]
                    if q == 0:
                        nc.vector.tensor_copy(ys0, psA[:])
                        nc.vector.tensor_copy(ys1, psB[:])
                    else:
                        nc.vector.tensor_add(ys0, ys0, psA[:])
                        nc.vector.tensor_add(ys1, ys1, psB[:])
                        if q == NQ - 1:
                            # split per 512-col half across two queues so
                            # the final stores drain in parallel
                            nc.sync.dma_start(
                                y_v[:, tq, 0:512], y[:, tq, 0:512]
                            )
                            nc.scalar.dma_start(
                                y_v[:, tq, 512:1024], y[:, tq, 512:1024]
                            )

                if q + 2 < NQ:
                    w2c = w2n
                    w2n = load_w2q(q + 2)
                elif q + 1 < NQ:
                    w2c = w2n

    nc.compile()
    return nc


def _get_nc(C: int):
    if C not in _BUILD_CACHE:
        _BUILD_CACHE[C] = build_nc(C)
    return _BUILD_CACHE[C]


def kernel(x, W1, b1, W2, b2, assign, k, _want_trace=False):
    x = np.asarray(x, dtype=np.float32)
    W1 = np.asarray(W1, dtype=np.float32)
    b1 = np.asarray(b1, dtype=np.float32)
    W2 = np.asarray(W2, dtype=np.float32)
    b2 = np.asarray(b2, dtype=np.float32)
    assign = np.asarray(assign)
    kk = int(k)

    assert W1.shape[0] == E and W2.shape[0] == E, "expert count must be 8"
    Bx, Sx, Dx = x.shape
    T = Bx * Sx
    x_bf = np.ascontiguousarray(x.reshape(T, Dx)).astype(BF16)
    a2 = assign.reshape(T, -1)

    idx = [np.nonzero((a2 == e).any(axis=1))[0] for e in range(E)]
    max_n = max(len(i) for i in idx)

    # capacity per device pass (multiple of 512); single pass for the
    # expected distribution, multiple passes if pathologically skewed
    C = min(max(1024, math.ceil(max_n / 512) * 512), 1536)
    n_pass = math.ceil(max(max_n, 1) / C)

    nc = _get_nc(C)

    w1_io = [
        np.ascontiguousarray(
            W1[e]
            .reshape(KT, 128, NQ, HTQ, 128)
            .transpose(1, 2, 3, 0, 4)
            .astype(BF16)
            .reshape(128, -1)
        )
        for e in range(E)
    ]
    w2_io = [
        np.ascontiguousarray(
            W2[e]
            .reshape(NQ, HTQ, 128, Dx)
            .transpose(2, 0, 1, 3)
            .astype(BF16)
            .reshape(128, -1)
        )
        for e in range(E)
    ]

    out_f = np.zeros((T, Dx), dtype=np.float32)
    trace_info = None

    for p in range(n_pass):
        in_maps = []
        for e in range(E):
            sl = idx[e][p * C : (p + 1) * C]
            xt_buf = np.zeros((128, KT, C), dtype=BF16)
            if len(sl):
                g = x_bf[sl]  # [n, D]
                xt_buf[:, :, : len(sl)] = g.reshape(len(sl), KT, 128).transpose(
                    2, 1, 0
                )
            in_maps.append(
                {
                    "xt": xt_buf.reshape(128, KT * C),
                    "w1": w1_io[e],
                    "b1": b1[e],
                    "w2": w2_io[e],
                }
            )
        res = run_bass_kernel_spmd(
            nc,
            in_maps,
            core_ids=list(range(NCORES)),
            trace=_want_trace,
            trace_cores=list(range(NCORES)) if _want_trace else None,
        )
        if _want_trace:
            trace_info = res
        for e in range(E):
            sl = idx[e][p * C : (p + 1) * C]
            if len(sl):
                out_f[sl] += res.results[e]["y"][: len(sl)] + b2[e][None, :]

    out = (out_f * np.float32(1.0 / kk)).reshape(Bx, Sx, Dx)
    if _want_trace:
        return out, trace_info
    return out

